# revision 2
# baseline (speedup 1.0000x reference)
"""Trainium kernel for nn_MlpAttention: 8-way batch-parallel MLP-attention scan.

Strategy (per sharding hint): data-parallel over batch B=16 across 8 NeuronCores
(2 batch elements per core), small weights replicated. Inside each core the
computation is algebraically restructured (exact up to fp reassociation):

  1. The per-step einsum  mem = cumsum(alpha*data) @ w_m.T  is hoisted out of
     the scan:  datam = data @ w_m.T  is computed ONCE, and the per-step memory
     projection becomes a cheap shifted cumsum of alpha*datam
     (linearity of the projection through the cumulative sum).
     This removes 13.4 GFLOP/step -> 13 MFLOP/step.
  2. att_h = data @ w_i2h.T stays hoisted (as in the reference).
  3. The heavy LSTM input GEMM (embed @ w_ih.T) is precomputed for all 100
     steps in one batched GEMM before the scan.
  4. contexts (ctx_u = sum_t beta_u[t] * data[t]) are deferred and computed
     after the scan as one batched GEMM over all steps.

Everything runs in float32.
"""

import numpy as np

NUM_CLASS = 5000
INPUT_DIM = 1024
EMB = 512
ATT = 512
CELL = 1024
OUT = 512
CLIP = 1.0
T, B, U = 800, 16, 100
N_CORES = 8

_COMPILED = {}


def _build():
    import jax
    import jax.numpy as jnp
    from functools import partial

    def core_fn(data, gx, att_h, datam, mask_pen, w_hh, b_lstm, w_proj,
                w_p2s, w_att_v, conv1_w, conv1_b, conv2_w, conv2_b):
        # data:[T,b,D] gx:[U,b,4C] att_h/datam:[T,b,A] mask_pen:[T,b]
        Tn = data.shape[0]
        bn = data.shape[1]

        c0 = jnp.zeros((bn, CELL), jnp.float32)
        p0 = jnp.zeros((bn, OUT), jnp.float32)
        lss0 = jnp.zeros((bn, Tn), jnp.float32)

        def step(carry, gx_u):
            c, p, lss = carry
            g = gx_u + p @ w_hh.T + b_lstm
            i, f, gg, o = jnp.split(g, 4, axis=-1)
            c = jax.nn.sigmoid(f) * c + jax.nn.sigmoid(i) * jnp.tanh(gg)
            c = jnp.clip(c, -CLIP, CLIP)
            p = (jax.nn.sigmoid(o) * jnp.tanh(c)) @ w_proj.T

            # location features: conv1 (k=21, 8ch) then 1x1 conv to ATT
            x = lss[:, None, :]                                   # [b,1,T]
            f1 = jax.lax.conv_general_dilated(
                x, conv1_w, (1,), [(10, 10)],
                dimension_numbers=("NCH", "OIH", "NCH"))          # [b,8,T]
            f1 = f1 + conv1_b[None, :, None]
            floc = jnp.einsum("bct,ac->tba", f1, conv2_w[:, :, 0])  # [T,b,A]
            floc = floc + conv2_b[None, None, :]

            state = p @ w_p2s.T                                   # [b,A]
            base = state[None] + att_h + floc                     # [T,b,A]
            s1 = jnp.sum(w_att_v * jnp.tanh(base), axis=-1) + mask_pen
            alpha = jax.nn.sigmoid(s1)                            # [T,b]
            # hoisted memory projection: shifted cumsum of alpha*datam
            mem = jnp.cumsum(alpha[..., None] * datam, axis=0)
            mem = jnp.concatenate([jnp.zeros_like(mem[:1]), mem[:-1]], axis=0)
            beta = jax.nn.sigmoid(
                jnp.sum(w_att_v * jnp.tanh(base + mem), axis=-1) + mask_pen)
            lss = lss + beta.T
            return (c, p, lss), (p, beta.T)

        _, (lstmps, alps) = jax.lax.scan(step, (c0, p0, lss0), gx)
        # deferred contexts: ctx[u,b,:] = sum_t alps[u,b,t] * data[t,b,:]
        contexts = jnp.einsum("ubt,tbi->ubi", alps, data)
        return contexts, lstmps, alps

    pcore = jax.pmap(core_fn, axis_name="x",
                     in_axes=(2, 2, 2, 2, 2, None, None, None, None, None,
                              None, None, None, None))

    @partial(jax.jit, static_argnums=())
    def prep(att_label, embed_table, w_ih, b_lstm, data, w_i2h, w_m, rnn_mask):
        sos = jnp.full((1, B), NUM_CLASS - 2, att_label.dtype)
        labels = jnp.concatenate([sos, att_label[:-1]], axis=0)   # [U,B]
        embed = embed_table[labels]                               # [U,B,EMB]
        gx = jnp.einsum("ube,ge->ubg", embed, w_ih)               # [U,B,4C]
        att_h = jnp.einsum("tbi,ai->tba", data, w_i2h)            # [T,B,A]
        datam = jnp.einsum("tbi,ai->tba", data, w_m)              # [T,B,A]
        mask_pen = (rnn_mask[:, :, 0] - 1.0) * 1e10               # [T,B]
        return gx, att_h, datam, mask_pen

    return pcore, prep


def kernel(data, att_mask, rnn_mask, att_label, embed_table, w_ih, w_hh,
           b_lstm, w_proj, w_i2h, w_p2s, w_m, w_att_v,
           conv1_w, conv1_b, conv2_w, conv2_b):
    import jax.numpy as jnp

    if "fns" not in _COMPILED:
        _COMPILED["fns"] = _build()
    pcore, prep = _COMPILED["fns"]

    data = jnp.asarray(data, jnp.float32)
    att_label = jnp.asarray(np.asarray(att_label).astype(np.int32))

    gx, att_h, datam, mask_pen = prep(
        att_label, jnp.asarray(embed_table), jnp.asarray(w_ih),
        jnp.asarray(b_lstm), data, jnp.asarray(w_i2h), jnp.asarray(w_m),
        jnp.asarray(rnn_mask))

    # shard batch across cores: [.., B, ..] -> [.., 8, 2, ..] with core axis
    def shard(x, axis):
        s = list(x.shape)
        s[axis:axis + 1] = [N_CORES, B // N_CORES]
        return jnp.reshape(x, s)

    data_s = shard(data, 1)          # [T,8,2,D]
    gx_s = shard(gx, 1)              # [U,8,2,4C]
    att_h_s = shard(att_h, 1)
    datam_s = shard(datam, 1)
    pen_s = shard(mask_pen, 1)       # [T,8,2]

    contexts, lstmps, alps = pcore(
        data_s, gx_s, att_h_s, datam_s, pen_s,
        jnp.asarray(w_hh), jnp.asarray(b_lstm), jnp.asarray(w_proj),
        jnp.asarray(w_p2s), jnp.asarray(w_att_v), jnp.asarray(conv1_w),
        jnp.asarray(conv1_b), jnp.asarray(conv2_w), jnp.asarray(conv2_b))
    # pmap maps axis 2 (size NDEV) of [.., 8grp, NDEV, ..]; device d holds
    # group-batches b = i*NDEV + d -> outputs [NDEV, U, 8grp, X]; un-shard by
    # moving dev axis innermost: [U, 8grp, NDEV, X] -> [U, B, X].
    def unshard(x, w):
        return np.asarray(x).transpose(1, 2, 0, 3).reshape(U, B, w)

    contexts = unshard(contexts, INPUT_DIM)
    lstmps = unshard(lstmps, OUT)
    alps = unshard(alps, T)

    m = np.asarray(att_mask)[..., None]
    return contexts * m, lstmps * m, alps
